# revision 8
# baseline (speedup 1.0000x reference)
"""Multi-head causal attention (B=2, S=2048, D=1024, H=16, dh=64) on 8
Trainium2 NeuronCores.

Sharding: core i handles batch b = i//4 and head group g = i%4 (4 heads
each).  Per core everything is computed in a transposed layout:

  QT = Wq_g^T @ x_b^T          [256(hk), 2048(S)]   (bf16)
  KT = Wk_g^T @ x_b^T          [256(hk), 2048(S)]   (bf16)
  V  = x_b @ Wv_g              [2048(S), 4, 65]     (bf16; col 64 = ones)
  per chunk c (512 queries), head-pair hp, key block j (128 keys):
     scT[par] = KT_h[:,j]^T(lhsT) x QT_h[:,c]   -> PSUM [128, 2, 512]
                (the two heads of a pair use PE rows 0-63 / 64-127 and
                 run concurrently)
     expT     = exp(scT/8) (* causal mask when j >= 4c)        (bf16)
     zT_h    += V_aug[j]^T(lhsT) x expT[par]    -> PSUM [65, 512]
                (row 64 accumulates the softmax denominator s)
     ztn      = zT[0:64] * broadcast(1/s)       [256(hk), 2048] (bf16)
  outT = Wo_g^T(lhsT) x ztn                     [1024(d), 2048] (bf16)

Host: shards/transposes inputs, sums the 4 head-group partial outputs per
batch, adds b_O and the exact b_V fold (softmax rows sum to 1):
  out += b_O + sum_h b_V[h] @ W_O[h].
"""
import numpy as np
import ml_dtypes

import concourse.bacc as bacc
import concourse.mybir as mybir
import concourse.tile as tile
from concourse.bass_utils import run_bass_kernel_spmd

f32 = mybir.dt.float32
bf16 = mybir.dt.bfloat16
AF = mybir.ActivationFunctionType

B, S, D, H, DH = 2, 2048, 1024, 16, 64
NCORES = 8
HG = 4                # heads per core
HK = HG * DH          # 256
CH = 512              # query chunk
NCH = S // CH         # 4
KB = 128              # key block
DT = D // 128         # 8

_CACHE = {}


def _build_nc():
    nc = bacc.Bacc(None, target_bir_lowering=False, debug=False,
                   num_devices=NCORES)

    xt_d = nc.dram_tensor("xt", [128, DT, S], bf16, kind="ExternalInput")
    wq_d = nc.dram_tensor("wq", [128, DT, HK], bf16, kind="ExternalInput")
    wk_d = nc.dram_tensor("wk", [128, DT, HK], bf16, kind="ExternalInput")
    wv_d = nc.dram_tensor("wv", [128, DT, HK], bf16, kind="ExternalInput")
    wo_d = nc.dram_tensor("wo", [128, 2, D], bf16, kind="ExternalInput")
    bq_d = nc.dram_tensor("bq", [128, 2], f32, kind="ExternalInput")
    bk_d = nc.dram_tensor("bk", [128, 2], f32, kind="ExternalInput")
    mask_d = nc.dram_tensor("mask", [128, 4, CH], bf16,
                            kind="ExternalInput")
    out_d = nc.dram_tensor("outT", [D, S], bf16, kind="ExternalOutput")

    with tile.TileContext(nc) as tc:
        with (
            tc.tile_pool(name="const", bufs=1) as cp,
            tc.tile_pool(name="big", bufs=1) as bp,
            tc.tile_pool(name="work", bufs=3) as wp,
            tc.tile_pool(name="psum", bufs=2, space="PSUM") as pp,
        ):
            # ---- loads
            wq = cp.tile([128, DT, HK], bf16)
            wk = cp.tile([128, DT, HK], bf16)
            wv = cp.tile([128, DT, HK], bf16)
            wo = cp.tile([128, 2, D], bf16)
            bq = cp.tile([128, 2], f32)
            bk = cp.tile([128, 2], f32)
            mask = cp.tile([128, 4, CH], bf16)
            xt = bp.tile([128, DT, S], bf16)
            # order DMAs so the first Q-projection matmul chain (needs wq +
            # xt tiles in di order) can start as early as possible
            nc.sync.dma_start(wq, wq_d[:])
            nc.sync.dma_start(xt[:, 0, :], xt_d[:, 0, :])
            nc.sync.dma_start(wk, wk_d[:])
            nc.sync.dma_start(xt[:, 1, :], xt_d[:, 1, :])
            nc.sync.dma_start(wv, wv_d[:])
            for di in range(2, DT):
                nc.sync.dma_start(xt[:, di, :], xt_d[:, di, :])
            nc.sync.dma_start(bq, bq_d[:])
            nc.sync.dma_start(bk, bk_d[:])
            nc.sync.dma_start(mask, mask_d[:])
            nc.sync.dma_start(wo, wo_d[:])

            qt = bp.tile([128, 2, S], bf16)
            kt = bp.tile([128, 2, S], bf16)
            v = bp.tile([128, S // KB, HG, DH + 1], bf16)
            ztn = bp.tile([128, 2, S], bf16)

            # ones column for the denominator trick
            nc.gpsimd.memset(v[:, :, :, DH:DH + 1], 1.0)

            # ---- phase B: projections (interleaved per chunk c)
            for c in range(NCH):
                cs = c * CH
                for m in range(2):
                    ps_q = pp.tile([128, CH], f32, tag="proj")
                    for di in range(DT):
                        nc.tensor.matmul(
                            ps_q, wq[:, di, m * 128:(m + 1) * 128],
                            xt[:, di, cs:cs + CH],
                            start=(di == 0), stop=(di == DT - 1))
                    nc.scalar.activation(qt[:, m, cs:cs + CH], ps_q,
                                         AF.Identity, bias=bq[:, m:m + 1])
                    ps_k = pp.tile([128, CH], f32, tag="proj")
                    for di in range(DT):
                        nc.tensor.matmul(
                            ps_k, wk[:, di, m * 128:(m + 1) * 128],
                            xt[:, di, cs:cs + CH],
                            start=(di == 0), stop=(di == DT - 1))
                    nc.scalar.activation(kt[:, m, cs:cs + CH], ps_k,
                                         AF.Identity, bias=bk[:, m:m + 1])
                for si in range(4 * c, 4 * c + 4):
                    ps_v = pp.tile([128, HG, DH], f32, tag="proj")
                    for di in range(DT):
                        nc.tensor.matmul(
                            ps_v, xt[:, di, si * KB:(si + 1) * KB],
                            wv[:, di, :],
                            start=(di == 0), stop=(di == DT - 1))
                    nc.vector.tensor_copy(v[:, si, :, 0:DH], ps_v)

            # ---- phase C + D interleaved per chunk
            for c in range(NCH):
                cs = c * CH
                nblk = 4 * c + 4       # key blocks for this chunk
                ngrp = nblk // 2       # groups of 2 key blocks
                for h in range(HG):
                    m, o = h // 2, (h % 2) * 64
                    zt = pp.tile([DH + 1, CH], f32, tag="zt",
                                 name=f"zt_{c}_{h}")
                    for g in range(ngrp):
                        sc = pp.tile([128, 2, CH], f32, tag="sc")
                        for u in range(2):
                            j = 2 * g + u
                            nc.tensor.matmul(
                                sc[:, u, :],
                                kt[o:o + 64, m, j * KB:(j + 1) * KB],
                                qt[o:o + 64, m, cs:cs + CH],
                                start=True, stop=True)
                        ex = wp.tile([128, 2, CH], bf16, tag="ex", bufs=6)
                        nc.scalar.activation(ex, sc, AF.Exp, scale=0.125)
                        if g >= ngrp - 2:
                            t0 = (g - (ngrp - 2)) * 2
                            nc.vector.tensor_mul(ex, ex,
                                                 mask[:, t0:t0 + 2, :])
                        for u in range(2):
                            j = 2 * g + u
                            nc.tensor.matmul(
                                zt, v[:, j, h, :], ex[:, u, :],
                                start=(j == 0), stop=(j == nblk - 1))
                    # normalize: ztn[h] = zt[0:64] / zt[64]
                    srow = wp.tile([1, CH], f32, tag="srow", bufs=3,
                                   name=f"srow_{c}_{h}")
                    nc.vector.tensor_copy(srow, zt[64:65, :])
                    rec = wp.tile([1, CH], f32, tag="rec", bufs=3,
                                  name=f"rec_{c}_{h}")
                    nc.vector.reciprocal_approx_fast(rec, srow)
                    bc = wp.tile([64, CH], f32, tag="bc", bufs=3,
                                 name=f"bc_{c}_{h}")
                    nc.gpsimd.partition_broadcast(bc, rec)
                    nc.vector.tensor_mul(ztn[o:o + 64, m, cs:cs + CH],
                                         zt[0:64, :], bc)

                # ---- output projection for this chunk
                for dt_i in range(DT):
                    ps_o = pp.tile([128, CH], f32, tag="proj",
                                   name=f"ps_o_{c}_{dt_i}")
                    for m in range(2):
                        nc.tensor.matmul(
                            ps_o, wo[:, m, dt_i * 128:(dt_i + 1) * 128],
                            ztn[:, m, cs:cs + CH],
                            start=(m == 0), stop=(m == 1))
                    ost = wp.tile([128, CH], bf16, tag="ost", bufs=4)
                    nc.vector.tensor_copy(ost, ps_o)
                    nc.sync.dma_start(
                        out_d[dt_i * 128:(dt_i + 1) * 128, cs:cs + CH], ost)

    nc.compile()
    return nc


def _tile128(a, inner_shape):
    """[N*128, ...] -> [128, N, ...] partition-major layout."""
    n = a.shape[0] // 128
    return np.ascontiguousarray(
        a.reshape((n, 128) + a.shape[1:]).swapaxes(0, 1)).reshape(
            (128, n) + inner_shape)


def _prep_core(x, W_Q, W_K, W_V, W_O, b_Q, b_K, b, g):
    hs = slice(g * HG, (g + 1) * HG)
    bfl = ml_dtypes.bfloat16

    xtp = np.ascontiguousarray(x[b].T)                       # [D, S]
    xt = _tile128(xtp, (S,)).astype(bfl)                     # [128, DT, S]

    def prep_w(w):                                           # [H,D,dh] slice
        wc = np.ascontiguousarray(
            w[hs].transpose(1, 0, 2).reshape(D, HK))         # [D, HK]
        return _tile128(wc, (HK,)).astype(bfl)               # [128, DT, HK]

    wq, wk, wv = prep_w(W_Q), prep_w(W_K), prep_w(W_V)
    woc = W_O[hs].reshape(HK, D)                             # [HK, D]
    wo = _tile128(woc, (D,)).astype(bfl)                     # [128, 2, D]

    bq = np.ascontiguousarray(
        b_Q[hs].reshape(HK).reshape(2, 128).T).astype(np.float32)
    bk = np.ascontiguousarray(
        b_K[hs].reshape(HK).reshape(2, 128).T).astype(np.float32)

    r = np.arange(128)[:, None, None]
    f = np.arange(CH)[None, None, :]
    t = np.arange(4)[None, :, None]
    mask = (f >= r + 128 * t).astype(bfl)                    # [128, 4, CH]

    return {"xt": xt, "wq": wq, "wk": wk, "wv": wv, "wo": wo,
            "bq": bq, "bk": bk, "mask": mask}


def kernel(x, W_Q, W_K, W_V, W_O, b_Q, b_K, b_V, b_O, **run_kwargs):
    x = np.asarray(x, dtype=np.float32)
    W_Q = np.asarray(W_Q, dtype=np.float32)
    W_K = np.asarray(W_K, dtype=np.float32)
    W_V = np.asarray(W_V, dtype=np.float32)
    W_O = np.asarray(W_O, dtype=np.float32)
    b_Q = np.asarray(b_Q, dtype=np.float32)
    b_K = np.asarray(b_K, dtype=np.float32)
    b_V = np.asarray(b_V, dtype=np.float32)
    b_O = np.asarray(b_O, dtype=np.float32)

    if "nc" not in _CACHE:
        _CACHE["nc"] = _build_nc()
    nc = _CACHE["nc"]

    in_maps = []
    for i in range(NCORES):
        b, g = i // HG, i % HG
        in_maps.append(_prep_core(x, W_Q, W_K, W_V, W_O, b_Q, b_K, b, g))

    res = run_bass_kernel_spmd(nc, in_maps, core_ids=list(range(NCORES)),
                               **run_kwargs)

    # exact fold of b_V through W_O (softmax rows sum to 1), plus b_O
    bias = (b_O.astype(np.float64)
            + b_V.reshape(H * DH).astype(np.float64)
            @ W_O.reshape(H * DH, D).astype(np.float64)).astype(np.float32)

    out = np.zeros((B, S, D), dtype=np.float32)
    for i in range(NCORES):
        b = i // HG
        out[b] += res.results[i]["outT"].astype(np.float32).T
    out += bias[None, None, :]
    if run_kwargs:
        return out, res
    return out


# revision 29
# speedup vs baseline: 1.1321x; 1.1321x over previous
"""Multi-head causal attention (B=2, S=2048, D=1024, H=16, dh=64) on 8
Trainium2 NeuronCores.

Sharding: core i handles batch b = i//4 and head group g = i%4 (4 heads
each).  Per core everything is computed in a transposed layout:

  QT = Wq_g^T @ x_b^T          [256(hk), 2048(S)]   (bf16)
  KT = Wk_g^T @ x_b^T          [256(hk), 2048(S)]   (bf16)
  V  = x_b @ Wv_g              [2048(S), 4, 65]     (bf16; col 64 = ones)
  per chunk c (512 queries), head-pair hp, key block j (128 keys):
     scT[par] = KT_h[:,j]^T(lhsT) x QT_h[:,c]   -> PSUM [128, 2, 512]
                (the two heads of a pair use PE rows 0-63 / 64-127 and
                 run concurrently)
     expT     = exp(scT/8) (* causal mask when j >= 4c)        (bf16)
     zT_h    += V_aug[j]^T(lhsT) x expT[par]    -> PSUM [65, 512]
                (row 64 accumulates the softmax denominator s)
     ztn      = zT[0:64] * broadcast(1/s)       [256(hk), 2048] (bf16)
  outT = Wo_g^T(lhsT) x ztn                     [1024(d), 2048] (bf16)

Host: shards/transposes inputs, sums the 4 head-group partial outputs per
batch, adds b_O and the exact b_V fold (softmax rows sum to 1):
  out += b_O + sum_h b_V[h] @ W_O[h].
"""
import numpy as np
import ml_dtypes

import concourse.bacc as bacc
import concourse.mybir as mybir
import concourse.tile as tile
from concourse.bass_utils import run_bass_kernel_spmd

f32 = mybir.dt.float32
bf16 = mybir.dt.bfloat16
AF = mybir.ActivationFunctionType

B, S, D, H, DH = 2, 2048, 1024, 16, 64
NCORES = 8
HG = 4                # heads per core
HK = HG * DH          # 256
CH = 512              # query chunk
NCH = S // CH         # 4
KB = 128              # key block
DT = D // 128         # 8

_CACHE = {}


def _build_nc():
    nc = bacc.Bacc(None, target_bir_lowering=False, debug=False,
                   num_devices=NCORES)

    xt_d = nc.dram_tensor("xt", [128, DT, S], bf16, kind="ExternalInput")
    wq_d = nc.dram_tensor("wq", [128, DT, HK], bf16, kind="ExternalInput")
    wk_d = nc.dram_tensor("wk", [128, DT, HK], bf16, kind="ExternalInput")
    wv_d = nc.dram_tensor("wv", [128, DT, HK], bf16, kind="ExternalInput")
    wo_d = nc.dram_tensor("wo", [128, 2, D], bf16, kind="ExternalInput")
    bq_d = nc.dram_tensor("bq", [128, 2], f32, kind="ExternalInput")
    bk_d = nc.dram_tensor("bk", [128, 2], f32, kind="ExternalInput")
    mask_d = nc.dram_tensor("mask", [128, 4, 2, CH], bf16,
                            kind="ExternalInput")
    out_d = nc.dram_tensor("outT", [D, S], bf16, kind="ExternalOutput")

    with tile.TileContext(nc) as tc:
        with (
            tc.tile_pool(name="const", bufs=1) as cp,
            tc.tile_pool(name="big", bufs=1) as bp,
            tc.tile_pool(name="work", bufs=3) as wp,
            tc.tile_pool(name="psum", bufs=2, space="PSUM") as pp,
        ):
            # ---- loads
            wq = cp.tile([128, DT, HK], bf16)
            wk = cp.tile([128, DT, HK], bf16)
            wv = cp.tile([128, DT, HK], bf16)
            wo = cp.tile([128, 2, D], bf16)
            bq = cp.tile([128, 2], f32)
            bk = cp.tile([128, 2], f32)
            mask = cp.tile([128, 4, 2, CH], bf16)
            xt = bp.tile([128, DT, S], bf16)
            # xt streams on the HWDGE ring (sync); weights and constants go
            # through the SWDGE ring (gpsimd) so the two transfer in parallel
            # and the first projection chains can start as early as possible
            nc.gpsimd.dma_start(wq, wq_d[:])
            for di in range(DT):
                nc.sync.dma_start(xt[:, di, 0:S // 2], xt_d[:, di, 0:S // 2])
                nc.sync.dma_start(xt[:, di, S // 2:], xt_d[:, di, S // 2:])
            nc.gpsimd.dma_start(wk, wk_d[:])
            nc.gpsimd.dma_start(wv, wv_d[:])
            nc.gpsimd.dma_start(bq, bq_d[:])
            nc.gpsimd.dma_start(bk, bk_d[:])
            nc.gpsimd.dma_start(mask, mask_d[:])
            nc.gpsimd.dma_start(wo, wo_d[:])

            qt = bp.tile([128, 2, S], bf16)
            kt = bp.tile([128, 2, S], bf16)
            # V padded to 128 columns (cols 65.. zero) so the zT matmul's
            # stationary is 128-wide -> fast weight load / ldw pipelining
            v = bp.tile([128, S // KB, HG, 128], bf16)
            ztn = bp.tile([128, 2, S], bf16)

            # ones column for the denominator trick; zero the pad
            nc.gpsimd.memset(v[:, :, :, DH:DH + 1], 1.0)
            nc.gpsimd.memset(v[:, :, :, DH + 1:], 0.0)

            # ---- phase B: projections (interleaved per chunk c).  During B
            # the attention psum tags are idle, so rotate B's psum tiles
            # through all tags to keep more projection chains in flight
            # while xt tiles stream in.
            btags = [("proj", 2), ("sc", 2), ("zt0", 1), ("zt1", 1)]
            bi = 0
            for c in range(NCH):
                cs = c * CH
                for m in range(2):
                    ps_q = pp.tile([128, CH], f32, tag=btags[bi % 4][0],
                                   bufs=btags[bi % 4][1], name=f"ps_q_{c}_{m}")
                    bi += 1
                    for di in range(DT):
                        nc.tensor.matmul(
                            ps_q, wq[:, di, m * 128:(m + 1) * 128],
                            xt[:, di, cs:cs + CH],
                            start=(di == 0), stop=(di == DT - 1))
                    nc.scalar.activation(qt[:, m, cs:cs + CH], ps_q,
                                         AF.Identity, bias=bq[:, m:m + 1])
                    ps_k = pp.tile([128, CH], f32, tag=btags[bi % 4][0],
                                   bufs=btags[bi % 4][1], name=f"ps_k_{c}_{m}")
                    bi += 1
                    for di in range(DT):
                        nc.tensor.matmul(
                            ps_k, wk[:, di, m * 128:(m + 1) * 128],
                            xt[:, di, cs:cs + CH],
                            start=(di == 0), stop=(di == DT - 1))
                    nc.scalar.activation(kt[:, m, cs:cs + CH], ps_k,
                                         AF.Identity, bias=bk[:, m:m + 1])
                for si in range(4 * c, 4 * c + 4):
                    ps_v = pp.tile([128, HG, DH], f32, tag=btags[bi % 4][0],
                                   bufs=btags[bi % 4][1], name=f"ps_v_{si}")
                    bi += 1
                    for di in range(DT):
                        nc.tensor.matmul(
                            ps_v, xt[:, di, si * KB:(si + 1) * KB],
                            wv[:, di, :],
                            start=(di == 0), stop=(di == DT - 1))
                    nc.vector.tensor_copy(v[:, si, :, 0:DH], ps_v)

            # ---- phase C: attention, head-pair row-packed scores
            for c in range(NCH):
                cs = c * CH
                nblk = 4 * c + 4       # key blocks for this chunk
                for hp in range(2):    # head pair (2hp, 2hp+1); m = hp
                    m = hp
                    zt0 = pp.tile([128, CH], f32, tag="zt0", bufs=1,
                                  name=f"zt0_{c}_{hp}")
                    zt1 = pp.tile([128, CH], f32, tag="zt1", bufs=1,
                                  name=f"zt1_{c}_{hp}")
                    zts = (zt0, zt1)
                    for j in range(nblk):
                        # diagonal blocks (t>=0): queries below 128t are
                        # fully masked -> compute only [128t, CH); the
                        # partially-masked region is just [128t, 128t+128)
                        t = j - 4 * c
                        ql = 128 * t if t > 0 else 0
                        sc = pp.tile([128, 2, CH], f32, tag="sc")
                        for par in range(2):
                            o = par * 64
                            nc.tensor.matmul(
                                sc[:, par, ql:],
                                kt[o:o + 64, m, j * KB:(j + 1) * KB],
                                qt[o:o + 64, m, cs + ql:cs + CH],
                                start=True, stop=True)
                        ex = wp.tile([128, 2, CH], bf16, tag="ex", bufs=6)
                        nc.scalar.activation(ex[:, :, ql:], sc[:, :, ql:],
                                             AF.Exp, scale=0.125)
                        if t >= 0:
                            qm = ql + 128
                            nc.vector.tensor_mul(ex[:, :, ql:qm],
                                                 ex[:, :, ql:qm],
                                                 mask[:, t, :, ql:qm])
                        for par in range(2):
                            h = 2 * hp + par
                            nc.tensor.matmul(
                                zts[par][:, ql:], v[:, j, h, :],
                                ex[:, par, ql:],
                                start=(j == 0), stop=(j == nblk - 1))
                    # normalize: ztn[h] = zt[0:64] / zt[64]
                    for par in range(2):
                        h = 2 * hp + par
                        o = par * 64
                        srow = wp.tile([1, CH], f32, tag="srow", bufs=3,
                                       name=f"srow_{c}_{h}")
                        nc.vector.tensor_copy(srow, zts[par][64:65, :])
                        rec = wp.tile([1, CH], f32, tag="rec", bufs=3,
                                      name=f"rec_{c}_{h}")
                        nc.vector.reciprocal_approx_fast(rec, srow)
                        bc = wp.tile([64, CH], f32, tag="bc", bufs=3,
                                     name=f"bc_{c}_{h}")
                        nc.gpsimd.partition_broadcast(bc, rec)
                        nc.vector.tensor_mul(ztn[o:o + 64, m, cs:cs + CH],
                                             zts[par][0:64, :], bc)

            # ---- phase D: output projection (emitted last; its matmuls
            # backfill PE idle slots during the ACT-paced tail of phase C)
            for c in range(NCH):
                cs = c * CH
                for dt_i in range(DT):
                    ps_o = pp.tile([128, CH], f32, tag="proj",
                                   name=f"ps_o_{c}_{dt_i}")
                    for m in range(2):
                        nc.tensor.matmul(
                            ps_o, wo[:, m, dt_i * 128:(dt_i + 1) * 128],
                            ztn[:, m, cs:cs + CH],
                            start=(m == 0), stop=(m == 1))
                    ost = wp.tile([128, CH], bf16, tag="ost", bufs=4)
                    nc.vector.tensor_copy(ost, ps_o)
                    nc.sync.dma_start(
                        out_d[dt_i * 128:(dt_i + 1) * 128, cs:cs + CH], ost)

    nc.compile()
    return nc


def _tile128(a, inner_shape):
    """[N*128, ...] -> [128, N, ...] partition-major layout."""
    n = a.shape[0] // 128
    return np.ascontiguousarray(
        a.reshape((n, 128) + a.shape[1:]).swapaxes(0, 1)).reshape(
            (128, n) + inner_shape)


def _prep_core(x, W_Q, W_K, W_V, W_O, b_Q, b_K, b, g):
    hs = slice(g * HG, (g + 1) * HG)
    bfl = ml_dtypes.bfloat16

    xtp = np.ascontiguousarray(x[b].T)                       # [D, S]
    xt = _tile128(xtp, (S,)).astype(bfl)                     # [128, DT, S]

    def prep_w(w):                                           # [H,D,dh] slice
        wc = np.ascontiguousarray(
            w[hs].transpose(1, 0, 2).reshape(D, HK))         # [D, HK]
        return _tile128(wc, (HK,)).astype(bfl)               # [128, DT, HK]

    wq, wk, wv = prep_w(W_Q), prep_w(W_K), prep_w(W_V)
    woc = W_O[hs].reshape(HK, D)                             # [HK, D]
    wo = _tile128(woc, (D,)).astype(bfl)                     # [128, 2, D]

    bq = np.ascontiguousarray(
        b_Q[hs].reshape(HK).reshape(2, 128).T).astype(np.float32)
    bk = np.ascontiguousarray(
        b_K[hs].reshape(HK).reshape(2, 128).T).astype(np.float32)

    r = np.arange(128)[:, None, None]
    f = np.arange(CH)[None, None, :]
    t = np.arange(4)[None, :, None]
    m3 = (f >= r + 128 * t)                                  # [128, 4, CH]
    mask = np.repeat(m3[:, :, None, :], 2, axis=2).astype(bfl)

    return {"xt": xt, "wq": wq, "wk": wk, "wv": wv, "wo": wo,
            "bq": bq, "bk": bk, "mask": mask}


def kernel(x, W_Q, W_K, W_V, W_O, b_Q, b_K, b_V, b_O, **run_kwargs):
    x = np.asarray(x, dtype=np.float32)
    W_Q = np.asarray(W_Q, dtype=np.float32)
    W_K = np.asarray(W_K, dtype=np.float32)
    W_V = np.asarray(W_V, dtype=np.float32)
    W_O = np.asarray(W_O, dtype=np.float32)
    b_Q = np.asarray(b_Q, dtype=np.float32)
    b_K = np.asarray(b_K, dtype=np.float32)
    b_V = np.asarray(b_V, dtype=np.float32)
    b_O = np.asarray(b_O, dtype=np.float32)

    if "nc" not in _CACHE:
        _CACHE["nc"] = _build_nc()
    nc = _CACHE["nc"]

    in_maps = []
    for i in range(NCORES):
        b, g = i // HG, i % HG
        in_maps.append(_prep_core(x, W_Q, W_K, W_V, W_O, b_Q, b_K, b, g))

    res = run_bass_kernel_spmd(nc, in_maps, core_ids=list(range(NCORES)),
                               **run_kwargs)

    # exact fold of b_V through W_O (softmax rows sum to 1), plus b_O
    bias = (b_O.astype(np.float64)
            + b_V.reshape(H * DH).astype(np.float64)
            @ W_O.reshape(H * DH, D).astype(np.float64)).astype(np.float32)

    out = np.zeros((B, S, D), dtype=np.float32)
    for i in range(NCORES):
        b = i // HG
        out[b] += res.results[i]["outT"].astype(np.float32).T
    out += bias[None, None, :]
    if run_kwargs:
        return out, res
    return out


# revision 31
# speedup vs baseline: 1.1640x; 1.0281x over previous
"""Multi-head causal attention (B=2, S=2048, D=1024, H=16, dh=64) on 8
Trainium2 NeuronCores.

Sharding: core i handles batch b = i//4 and head group g = i%4 (4 heads
each).  Per core everything is computed in a transposed layout:

  QT = Wq_g^T @ x_b^T          [256(hk), 2048(S)]   (bf16)
  KT = Wk_g^T @ x_b^T          [256(hk), 2048(S)]   (bf16)
  V  = x_b @ Wv_g              [2048(S), 4, 65]     (bf16; col 64 = ones)
  per chunk c (512 queries), head-pair hp, key block j (128 keys):
     scT[par] = KT_h[:,j]^T(lhsT) x QT_h[:,c]   -> PSUM [128, 2, 512]
                (the two heads of a pair use PE rows 0-63 / 64-127 and
                 run concurrently)
     expT     = exp(scT/8) (* causal mask when j >= 4c)        (bf16)
     zT_h    += V_aug[j]^T(lhsT) x expT[par]    -> PSUM [65, 512]
                (row 64 accumulates the softmax denominator s)
     ztn      = zT[0:64] * broadcast(1/s)       [256(hk), 2048] (bf16)
  outT = Wo_g^T(lhsT) x ztn                     [1024(d), 2048] (bf16)

Host: shards/transposes inputs, sums the 4 head-group partial outputs per
batch, adds b_O and the exact b_V fold (softmax rows sum to 1):
  out += b_O + sum_h b_V[h] @ W_O[h].
"""
import numpy as np
import ml_dtypes

import concourse.bacc as bacc
import concourse.mybir as mybir
import concourse.tile as tile
from concourse.bass_utils import run_bass_kernel_spmd

f32 = mybir.dt.float32
bf16 = mybir.dt.bfloat16
AF = mybir.ActivationFunctionType

B, S, D, H, DH = 2, 2048, 1024, 16, 64
NCORES = 8
HG = 4                # heads per core
HK = HG * DH          # 256
CH = 512              # query chunk
NCH = S // CH         # 4
KB = 128              # key block
DT = D // 128         # 8

_CACHE = {}


def _build_nc():
    nc = bacc.Bacc(None, target_bir_lowering=False, debug=False,
                   num_devices=NCORES)

    xt_d = nc.dram_tensor("xt", [128, DT, S], bf16, kind="ExternalInput")
    wq_d = nc.dram_tensor("wq", [128, DT, HK], bf16, kind="ExternalInput")
    wk_d = nc.dram_tensor("wk", [128, DT, HK], bf16, kind="ExternalInput")
    wv_d = nc.dram_tensor("wv", [128, DT, HK], bf16, kind="ExternalInput")
    wo_d = nc.dram_tensor("wo", [128, 2, D], bf16, kind="ExternalInput")
    bq_d = nc.dram_tensor("bq", [128, 2], f32, kind="ExternalInput")
    bk_d = nc.dram_tensor("bk", [128, 2], f32, kind="ExternalInput")
    mask_d = nc.dram_tensor("mask", [128, 4, 2, CH], bf16,
                            kind="ExternalInput")
    out_d = nc.dram_tensor("outT", [D, S], bf16, kind="ExternalOutput")

    with tile.TileContext(nc) as tc:
        with (
            tc.tile_pool(name="const", bufs=1) as cp,
            tc.tile_pool(name="big", bufs=1) as bp,
            tc.tile_pool(name="work", bufs=3) as wp,
            tc.tile_pool(name="psum", bufs=2, space="PSUM") as pp,
        ):
            # ---- loads
            wq = cp.tile([128, DT, HK], bf16)
            wk = cp.tile([128, DT, HK], bf16)
            wv = cp.tile([128, DT, HK], bf16)
            wo = cp.tile([128, 2, D], bf16)
            bq = cp.tile([128, 2], f32)
            bk = cp.tile([128, 2], f32)
            mask = cp.tile([128, 4, 2, CH], bf16)
            xt = bp.tile([128, DT, S], bf16)
            # xt streams on the HWDGE ring (sync); weights and constants go
            # through the SWDGE ring (gpsimd) so the two transfer in parallel
            # and the first projection chains can start as early as possible
            nc.sync.dma_start(wq, wq_d[:])
            for di in range(DT):
                nc.sync.dma_start(xt[:, di, 0:S // 2], xt_d[:, di, 0:S // 2])
            for di in range(DT):
                nc.sync.dma_start(xt[:, di, S // 2:], xt_d[:, di, S // 2:])
            nc.gpsimd.dma_start(wk, wk_d[:])
            nc.gpsimd.dma_start(wv, wv_d[:])
            nc.gpsimd.dma_start(bq, bq_d[:])
            nc.gpsimd.dma_start(bk, bk_d[:])
            nc.gpsimd.dma_start(mask, mask_d[:])
            nc.gpsimd.dma_start(wo, wo_d[:])

            qt = bp.tile([128, 2, S], bf16)
            kt = bp.tile([128, 2, S], bf16)
            # V padded to 128 columns (cols 65.. zero) so the zT matmul's
            # stationary is 128-wide -> fast weight load / ldw pipelining
            v = bp.tile([128, S // KB, HG, 128], bf16)
            ztn = bp.tile([128, 2, S], bf16)

            # ones column for the denominator trick; zero the pad
            nc.gpsimd.memset(v[:, :, :, DH:DH + 1], 1.0)
            nc.gpsimd.memset(v[:, :, :, DH + 1:], 0.0)

            # ---- phase B: projections (interleaved per chunk c).  During B
            # the attention psum tags are idle, so rotate B's psum tiles
            # through all tags to keep more projection chains in flight
            # while xt tiles stream in.
            btags = [("proj", 2), ("sc", 2), ("zt0", 1), ("zt1", 1)]
            bi = 0
            for c in range(NCH):
                cs = c * CH
                for m in range(2):
                    ps_q = pp.tile([128, CH], f32, tag=btags[bi % 4][0],
                                   bufs=btags[bi % 4][1], name=f"ps_q_{c}_{m}")
                    bi += 1
                    for di in range(DT):
                        nc.tensor.matmul(
                            ps_q, wq[:, di, m * 128:(m + 1) * 128],
                            xt[:, di, cs:cs + CH],
                            start=(di == 0), stop=(di == DT - 1))
                    nc.scalar.activation(qt[:, m, cs:cs + CH], ps_q,
                                         AF.Identity, bias=bq[:, m:m + 1])
                    ps_k = pp.tile([128, CH], f32, tag=btags[bi % 4][0],
                                   bufs=btags[bi % 4][1], name=f"ps_k_{c}_{m}")
                    bi += 1
                    for di in range(DT):
                        nc.tensor.matmul(
                            ps_k, wk[:, di, m * 128:(m + 1) * 128],
                            xt[:, di, cs:cs + CH],
                            start=(di == 0), stop=(di == DT - 1))
                    nc.scalar.activation(kt[:, m, cs:cs + CH], ps_k,
                                         AF.Identity, bias=bk[:, m:m + 1])
                for si in range(4 * c, 4 * c + 4):
                    ps_v = pp.tile([128, HG, DH], f32, tag=btags[bi % 4][0],
                                   bufs=btags[bi % 4][1], name=f"ps_v_{si}")
                    bi += 1
                    for di in range(DT):
                        nc.tensor.matmul(
                            ps_v, xt[:, di, si * KB:(si + 1) * KB],
                            wv[:, di, :],
                            start=(di == 0), stop=(di == DT - 1))
                    nc.vector.tensor_copy(v[:, si, :, 0:DH], ps_v)

            # ---- phase C: attention, head-pair row-packed scores
            for c in range(NCH):
                cs = c * CH
                nblk = 4 * c + 4       # key blocks for this chunk
                for hp in range(2):    # head pair (2hp, 2hp+1); m = hp
                    m = hp
                    zt0 = pp.tile([128, CH], f32, tag="zt0", bufs=1,
                                  name=f"zt0_{c}_{hp}")
                    zt1 = pp.tile([128, CH], f32, tag="zt1", bufs=1,
                                  name=f"zt1_{c}_{hp}")
                    zts = (zt0, zt1)
                    for j in range(nblk):
                        # diagonal blocks (t>=0): queries below 128t are
                        # fully masked -> compute only [128t, CH); the
                        # partially-masked region is just [128t, 128t+128)
                        t = j - 4 * c
                        ql = 128 * t if t > 0 else 0
                        sc = pp.tile([128, 2, CH], f32, tag="sc")
                        for par in range(2):
                            o = par * 64
                            nc.tensor.matmul(
                                sc[:, par, ql:],
                                kt[o:o + 64, m, j * KB:(j + 1) * KB],
                                qt[o:o + 64, m, cs + ql:cs + CH],
                                start=True, stop=True)
                        ex = wp.tile([128, 2, CH], bf16, tag="ex", bufs=6)
                        nc.scalar.activation(ex[:, :, ql:], sc[:, :, ql:],
                                             AF.Exp, scale=0.125)
                        if t >= 0:
                            qm = ql + 128
                            nc.vector.tensor_mul(ex[:, :, ql:qm],
                                                 ex[:, :, ql:qm],
                                                 mask[:, t, :, ql:qm])
                        for par in range(2):
                            h = 2 * hp + par
                            nc.tensor.matmul(
                                zts[par][:, ql:], v[:, j, h, :],
                                ex[:, par, ql:],
                                start=(j == 0), stop=(j == nblk - 1))
                    # normalize: ztn[h] = zt[0:64] / zt[64].  First copy
                    # the whole zT+denominator block to SBUF in one op so the
                    # PSUM accumulator frees immediately for the next head
                    # pair; the divide chain then runs entirely from SBUF.
                    for par in range(2):
                        h = 2 * hp + par
                        o = par * 64
                        zs = wp.tile([DH + 1, CH], f32, tag="zs", bufs=3,
                                     name=f"zs_{c}_{h}")
                        nc.vector.tensor_copy(zs, zts[par][0:DH + 1, :])
                        srow = wp.tile([1, CH], f32, tag="srow", bufs=3,
                                       name=f"srow_{c}_{h}")
                        nc.vector.tensor_copy(srow, zs[DH:DH + 1, :])
                        rec = wp.tile([1, CH], f32, tag="rec", bufs=3,
                                      name=f"rec_{c}_{h}")
                        nc.vector.reciprocal_approx_fast(rec, srow)
                        bc = wp.tile([64, CH], f32, tag="bc", bufs=3,
                                     name=f"bc_{c}_{h}")
                        nc.gpsimd.partition_broadcast(bc, rec)
                        nc.vector.tensor_mul(ztn[o:o + 64, m, cs:cs + CH],
                                             zs[0:DH, :], bc)

            # ---- phase D: output projection (emitted last; its matmuls
            # backfill PE idle slots during the ACT-paced tail of phase C)
            for c in range(NCH):
                cs = c * CH
                for dt_i in range(DT):
                    ps_o = pp.tile([128, CH], f32, tag="proj",
                                   name=f"ps_o_{c}_{dt_i}")
                    for m in range(2):
                        nc.tensor.matmul(
                            ps_o, wo[:, m, dt_i * 128:(dt_i + 1) * 128],
                            ztn[:, m, cs:cs + CH],
                            start=(m == 0), stop=(m == 1))
                    ost = wp.tile([128, CH], bf16, tag="ost", bufs=4)
                    nc.vector.tensor_copy(ost, ps_o)
                    nc.sync.dma_start(
                        out_d[dt_i * 128:(dt_i + 1) * 128, cs:cs + CH], ost)

    nc.compile()
    return nc


def _tile128(a, inner_shape):
    """[N*128, ...] -> [128, N, ...] partition-major layout."""
    n = a.shape[0] // 128
    return np.ascontiguousarray(
        a.reshape((n, 128) + a.shape[1:]).swapaxes(0, 1)).reshape(
            (128, n) + inner_shape)


def _prep_core(x, W_Q, W_K, W_V, W_O, b_Q, b_K, b, g):
    hs = slice(g * HG, (g + 1) * HG)
    bfl = ml_dtypes.bfloat16

    xtp = np.ascontiguousarray(x[b].T)                       # [D, S]
    xt = _tile128(xtp, (S,)).astype(bfl)                     # [128, DT, S]

    def prep_w(w):                                           # [H,D,dh] slice
        wc = np.ascontiguousarray(
            w[hs].transpose(1, 0, 2).reshape(D, HK))         # [D, HK]
        return _tile128(wc, (HK,)).astype(bfl)               # [128, DT, HK]

    wq, wk, wv = prep_w(W_Q), prep_w(W_K), prep_w(W_V)
    woc = W_O[hs].reshape(HK, D)                             # [HK, D]
    wo = _tile128(woc, (D,)).astype(bfl)                     # [128, 2, D]

    bq = np.ascontiguousarray(
        b_Q[hs].reshape(HK).reshape(2, 128).T).astype(np.float32)
    bk = np.ascontiguousarray(
        b_K[hs].reshape(HK).reshape(2, 128).T).astype(np.float32)

    r = np.arange(128)[:, None, None]
    f = np.arange(CH)[None, None, :]
    t = np.arange(4)[None, :, None]
    m3 = (f >= r + 128 * t)                                  # [128, 4, CH]
    mask = np.repeat(m3[:, :, None, :], 2, axis=2).astype(bfl)

    return {"xt": xt, "wq": wq, "wk": wk, "wv": wv, "wo": wo,
            "bq": bq, "bk": bk, "mask": mask}


def kernel(x, W_Q, W_K, W_V, W_O, b_Q, b_K, b_V, b_O, **run_kwargs):
    x = np.asarray(x, dtype=np.float32)
    W_Q = np.asarray(W_Q, dtype=np.float32)
    W_K = np.asarray(W_K, dtype=np.float32)
    W_V = np.asarray(W_V, dtype=np.float32)
    W_O = np.asarray(W_O, dtype=np.float32)
    b_Q = np.asarray(b_Q, dtype=np.float32)
    b_K = np.asarray(b_K, dtype=np.float32)
    b_V = np.asarray(b_V, dtype=np.float32)
    b_O = np.asarray(b_O, dtype=np.float32)

    if "nc" not in _CACHE:
        _CACHE["nc"] = _build_nc()
    nc = _CACHE["nc"]

    in_maps = []
    for i in range(NCORES):
        b, g = i // HG, i % HG
        in_maps.append(_prep_core(x, W_Q, W_K, W_V, W_O, b_Q, b_K, b, g))

    res = run_bass_kernel_spmd(nc, in_maps, core_ids=list(range(NCORES)),
                               **run_kwargs)

    # exact fold of b_V through W_O (softmax rows sum to 1), plus b_O
    bias = (b_O.astype(np.float64)
            + b_V.reshape(H * DH).astype(np.float64)
            @ W_O.reshape(H * DH, D).astype(np.float64)).astype(np.float32)

    out = np.zeros((B, S, D), dtype=np.float32)
    for i in range(NCORES):
        b = i // HG
        out[b] += res.results[i]["outT"].astype(np.float32).T
    out += bias[None, None, :]
    if run_kwargs:
        return out, res
    return out


# revision 32
# speedup vs baseline: 1.1726x; 1.0074x over previous
"""Multi-head causal attention (B=2, S=2048, D=1024, H=16, dh=64) on 8
Trainium2 NeuronCores.

Sharding: core i handles batch b = i//4 and head group g = i%4 (4 heads
each).  Per core everything is computed in a transposed layout:

  QT = Wq_g^T @ x_b^T          [256(hk), 2048(S)]   (bf16)
  KT = Wk_g^T @ x_b^T          [256(hk), 2048(S)]   (bf16)
  V  = x_b @ Wv_g              [2048(S), 4, 65]     (bf16; col 64 = ones)
  per chunk c (512 queries), head-pair hp, key block j (128 keys):
     scT[par] = KT_h[:,j]^T(lhsT) x QT_h[:,c]   -> PSUM [128, 2, 512]
                (the two heads of a pair use PE rows 0-63 / 64-127 and
                 run concurrently)
     expT     = exp(scT/8) (* causal mask when j >= 4c)        (bf16)
     zT_h    += V_aug[j]^T(lhsT) x expT[par]    -> PSUM [65, 512]
                (row 64 accumulates the softmax denominator s)
     ztn      = zT[0:64] * broadcast(1/s)       [256(hk), 2048] (bf16)
  outT = Wo_g^T(lhsT) x ztn                     [1024(d), 2048] (bf16)

Host: shards/transposes inputs, sums the 4 head-group partial outputs per
batch, adds b_O and the exact b_V fold (softmax rows sum to 1):
  out += b_O + sum_h b_V[h] @ W_O[h].
"""
import numpy as np
import ml_dtypes

import concourse.bacc as bacc
import concourse.mybir as mybir
import concourse.tile as tile
from concourse.bass_utils import run_bass_kernel_spmd

f32 = mybir.dt.float32
bf16 = mybir.dt.bfloat16
AF = mybir.ActivationFunctionType

B, S, D, H, DH = 2, 2048, 1024, 16, 64
NCORES = 8
HG = 4                # heads per core
HK = HG * DH          # 256
CH = 512              # query chunk
NCH = S // CH         # 4
KB = 128              # key block
DT = D // 128         # 8

_CACHE = {}


def _build_nc():
    nc = bacc.Bacc(None, target_bir_lowering=False, debug=False,
                   num_devices=NCORES)

    xt_d = nc.dram_tensor("xt", [128, DT, S], bf16, kind="ExternalInput")
    wq_d = nc.dram_tensor("wq", [128, DT, HK], bf16, kind="ExternalInput")
    wk_d = nc.dram_tensor("wk", [128, DT, HK], bf16, kind="ExternalInput")
    wv_d = nc.dram_tensor("wv", [128, DT, HK], bf16, kind="ExternalInput")
    wo_d = nc.dram_tensor("wo", [128, 2, D], bf16, kind="ExternalInput")
    bq_d = nc.dram_tensor("bq", [128, 2], f32, kind="ExternalInput")
    bk_d = nc.dram_tensor("bk", [128, 2], f32, kind="ExternalInput")
    mask_d = nc.dram_tensor("mask", [128, 4, 2, CH], bf16,
                            kind="ExternalInput")
    out_d = nc.dram_tensor("outT", [D, S], bf16, kind="ExternalOutput")

    with tile.TileContext(nc) as tc:
        with (
            tc.tile_pool(name="const", bufs=1) as cp,
            tc.tile_pool(name="big", bufs=1) as bp,
            tc.tile_pool(name="work", bufs=3) as wp,
            tc.tile_pool(name="psum", bufs=2, space="PSUM") as pp,
        ):
            # ---- loads
            wq = cp.tile([128, DT, HK], bf16)
            wk = cp.tile([128, DT, HK], bf16)
            wv = cp.tile([128, DT, HK], bf16)
            wo = cp.tile([128, 2, D], bf16)
            bq = cp.tile([128, 2], f32)
            bk = cp.tile([128, 2], f32)
            mask = cp.tile([128, 4, 2, CH], bf16)
            xt = bp.tile([128, DT, S], bf16)
            # xt streams on the HWDGE ring (sync); weights and constants go
            # through the SWDGE ring (gpsimd) so the two transfer in parallel
            # and the first projection chains can start as early as possible
            nc.sync.dma_start(wq, wq_d[:])
            for di in range(DT):
                nc.sync.dma_start(xt[:, di, 0:S // 2], xt_d[:, di, 0:S // 2])
            for di in range(DT):
                nc.sync.dma_start(xt[:, di, S // 2:], xt_d[:, di, S // 2:])
            nc.gpsimd.dma_start(wk, wk_d[:])
            nc.gpsimd.dma_start(wv, wv_d[:])
            nc.gpsimd.dma_start(bq, bq_d[:])
            nc.gpsimd.dma_start(bk, bk_d[:])
            nc.gpsimd.dma_start(mask, mask_d[:])
            nc.gpsimd.dma_start(wo, wo_d[:])

            qt = bp.tile([128, 2, S], bf16)
            kt = bp.tile([128, 2, S], bf16)
            # V padded to 128 columns (cols 65.. zero) so the zT matmul's
            # stationary is 128-wide -> fast weight load / ldw pipelining
            v = bp.tile([128, S // KB, HG, 128], bf16)
            ztn = bp.tile([128, 2, S], bf16)

            # ones column for the denominator trick; zero the pad
            nc.gpsimd.memset(v[:, :, :, DH:DH + 1], 1.0)
            nc.gpsimd.memset(v[:, :, :, DH + 1:], 0.0)

            # ---- phase B: projections (interleaved per chunk c).  During B
            # the attention psum tags are idle, so rotate B's psum tiles
            # through all tags to keep more projection chains in flight
            # while xt tiles stream in.
            btags = [("proj", 2), ("sc", 2), ("zt0", 1), ("zt1", 1)]
            bi = 0
            for c in range(NCH):
                cs = c * CH
                for m in range(2):
                    ps_q = pp.tile([128, CH], f32, tag=btags[bi % 4][0],
                                   bufs=btags[bi % 4][1], name=f"ps_q_{c}_{m}")
                    bi += 1
                    for di in range(DT):
                        nc.tensor.matmul(
                            ps_q, wq[:, di, m * 128:(m + 1) * 128],
                            xt[:, di, cs:cs + CH],
                            start=(di == 0), stop=(di == DT - 1))
                    nc.scalar.activation(qt[:, m, cs:cs + CH], ps_q,
                                         AF.Identity, bias=bq[:, m:m + 1])
                    ps_k = pp.tile([128, CH], f32, tag=btags[bi % 4][0],
                                   bufs=btags[bi % 4][1], name=f"ps_k_{c}_{m}")
                    bi += 1
                    for di in range(DT):
                        nc.tensor.matmul(
                            ps_k, wk[:, di, m * 128:(m + 1) * 128],
                            xt[:, di, cs:cs + CH],
                            start=(di == 0), stop=(di == DT - 1))
                    nc.scalar.activation(kt[:, m, cs:cs + CH], ps_k,
                                         AF.Identity, bias=bk[:, m:m + 1])
                for si in range(4 * c, 4 * c + 4):
                    ps_v = pp.tile([128, HG, DH], f32, tag=btags[bi % 4][0],
                                   bufs=btags[bi % 4][1], name=f"ps_v_{si}")
                    bi += 1
                    for di in range(DT):
                        nc.tensor.matmul(
                            ps_v, xt[:, di, si * KB:(si + 1) * KB],
                            wv[:, di, :],
                            start=(di == 0), stop=(di == DT - 1))
                    nc.vector.tensor_copy(v[:, si, :, 0:DH], ps_v)

            # ---- phase C: attention, head-pair row-packed scores
            for c in range(NCH):
                cs = c * CH
                nblk = 4 * c + 4       # key blocks for this chunk
                for hp in range(2):    # head pair (2hp, 2hp+1); m = hp
                    m = hp
                    zt0 = pp.tile([128, CH], f32, tag="zt0", bufs=1,
                                  name=f"zt0_{c}_{hp}")
                    zt1 = pp.tile([128, CH], f32, tag="zt1", bufs=1,
                                  name=f"zt1_{c}_{hp}")
                    zts = (zt0, zt1)
                    for j in range(nblk):
                        # diagonal blocks (t>=0): queries below 128t are
                        # fully masked -> compute only [128t, CH); the
                        # partially-masked region is just [128t, 128t+128)
                        t = j - 4 * c
                        ql = 128 * t if t > 0 else 0
                        sc = pp.tile([128, 2, CH], f32, tag="sc")
                        for par in range(2):
                            o = par * 64
                            nc.tensor.matmul(
                                sc[:, par, ql:],
                                kt[o:o + 64, m, j * KB:(j + 1) * KB],
                                qt[o:o + 64, m, cs + ql:cs + CH],
                                start=True, stop=True)
                        ex = wp.tile([128, 2, CH], bf16, tag="ex", bufs=6)
                        nc.scalar.activation(ex[:, :, ql:], sc[:, :, ql:],
                                             AF.Exp, scale=0.125)
                        if t >= 0:
                            qm = ql + 128
                            nc.vector.tensor_mul(ex[:, :, ql:qm],
                                                 ex[:, :, ql:qm],
                                                 mask[:, t, :, ql:qm])
                        for par in range(2):
                            h = 2 * hp + par
                            nc.tensor.matmul(
                                zts[par][:, ql:], v[:, j, h, :],
                                ex[:, par, ql:],
                                start=(j == 0), stop=(j == nblk - 1))
                    # normalize: ztn[h] = zt[0:64] / zt[64].  First copy
                    # the whole zT+denominator block to SBUF in one op so the
                    # PSUM accumulator frees immediately for the next head
                    # pair; the divide chain then runs entirely from SBUF.
                    for par in range(2):
                        h = 2 * hp + par
                        o = par * 64
                        zs = wp.tile([DH + 1, CH], f32, tag="zs", bufs=3,
                                     name=f"zs_{c}_{h}")
                        nc.vector.tensor_copy(zs, zts[par][0:DH + 1, :])
                        srow = wp.tile([1, CH], f32, tag="srow", bufs=3,
                                       name=f"srow_{c}_{h}")
                        nc.vector.tensor_copy(srow, zs[DH:DH + 1, :])
                        rec = wp.tile([1, CH], f32, tag="rec", bufs=3,
                                      name=f"rec_{c}_{h}")
                        nc.vector.reciprocal_approx_fast(rec, srow)
                        bc = wp.tile([64, CH], f32, tag="bc", bufs=3,
                                     name=f"bc_{c}_{h}")
                        nc.gpsimd.partition_broadcast(bc, rec)
                        nc.vector.tensor_mul(ztn[o:o + 64, m, cs:cs + CH],
                                             zs[0:DH, :], bc)

            # ---- phase D: output projection (emitted last; its matmuls
            # backfill PE idle slots during the ACT-paced tail of phase C).
            # The last chunk's tiles rotate through ALL psum tags: by then
            # phase C has released the sc/zt banks, so the final 8 output
            # tiles pipeline through 6 slots instead of 2.
            dtags = [("proj", 2), ("sc", 2), ("zt0", 1), ("zt1", 1)]
            for c in range(NCH):
                cs = c * CH
                for dt_i in range(DT):
                    tg, tb = dtags[dt_i % 4] if c == NCH - 1 else dtags[0]
                    ps_o = pp.tile([128, CH], f32, tag=tg, bufs=tb,
                                   name=f"ps_o_{c}_{dt_i}")
                    for m in range(2):
                        nc.tensor.matmul(
                            ps_o, wo[:, m, dt_i * 128:(dt_i + 1) * 128],
                            ztn[:, m, cs:cs + CH],
                            start=(m == 0), stop=(m == 1))
                    ost = wp.tile([128, CH], bf16, tag="ost", bufs=4)
                    nc.vector.tensor_copy(ost, ps_o)
                    nc.sync.dma_start(
                        out_d[dt_i * 128:(dt_i + 1) * 128, cs:cs + CH], ost)

    nc.compile()
    return nc


def _tile128(a, inner_shape):
    """[N*128, ...] -> [128, N, ...] partition-major layout."""
    n = a.shape[0] // 128
    return np.ascontiguousarray(
        a.reshape((n, 128) + a.shape[1:]).swapaxes(0, 1)).reshape(
            (128, n) + inner_shape)


def _prep_core(x, W_Q, W_K, W_V, W_O, b_Q, b_K, b, g):
    hs = slice(g * HG, (g + 1) * HG)
    bfl = ml_dtypes.bfloat16

    xtp = np.ascontiguousarray(x[b].T)                       # [D, S]
    xt = _tile128(xtp, (S,)).astype(bfl)                     # [128, DT, S]

    def prep_w(w):                                           # [H,D,dh] slice
        wc = np.ascontiguousarray(
            w[hs].transpose(1, 0, 2).reshape(D, HK))         # [D, HK]
        return _tile128(wc, (HK,)).astype(bfl)               # [128, DT, HK]

    wq, wk, wv = prep_w(W_Q), prep_w(W_K), prep_w(W_V)
    woc = W_O[hs].reshape(HK, D)                             # [HK, D]
    wo = _tile128(woc, (D,)).astype(bfl)                     # [128, 2, D]

    bq = np.ascontiguousarray(
        b_Q[hs].reshape(HK).reshape(2, 128).T).astype(np.float32)
    bk = np.ascontiguousarray(
        b_K[hs].reshape(HK).reshape(2, 128).T).astype(np.float32)

    r = np.arange(128)[:, None, None]
    f = np.arange(CH)[None, None, :]
    t = np.arange(4)[None, :, None]
    m3 = (f >= r + 128 * t)                                  # [128, 4, CH]
    mask = np.repeat(m3[:, :, None, :], 2, axis=2).astype(bfl)

    return {"xt": xt, "wq": wq, "wk": wk, "wv": wv, "wo": wo,
            "bq": bq, "bk": bk, "mask": mask}


def kernel(x, W_Q, W_K, W_V, W_O, b_Q, b_K, b_V, b_O, **run_kwargs):
    x = np.asarray(x, dtype=np.float32)
    W_Q = np.asarray(W_Q, dtype=np.float32)
    W_K = np.asarray(W_K, dtype=np.float32)
    W_V = np.asarray(W_V, dtype=np.float32)
    W_O = np.asarray(W_O, dtype=np.float32)
    b_Q = np.asarray(b_Q, dtype=np.float32)
    b_K = np.asarray(b_K, dtype=np.float32)
    b_V = np.asarray(b_V, dtype=np.float32)
    b_O = np.asarray(b_O, dtype=np.float32)

    if "nc" not in _CACHE:
        _CACHE["nc"] = _build_nc()
    nc = _CACHE["nc"]

    in_maps = []
    for i in range(NCORES):
        b, g = i // HG, i % HG
        in_maps.append(_prep_core(x, W_Q, W_K, W_V, W_O, b_Q, b_K, b, g))

    res = run_bass_kernel_spmd(nc, in_maps, core_ids=list(range(NCORES)),
                               **run_kwargs)

    # exact fold of b_V through W_O (softmax rows sum to 1), plus b_O
    bias = (b_O.astype(np.float64)
            + b_V.reshape(H * DH).astype(np.float64)
            @ W_O.reshape(H * DH, D).astype(np.float64)).astype(np.float32)

    out = np.zeros((B, S, D), dtype=np.float32)
    for i in range(NCORES):
        b = i // HG
        out[b] += res.results[i]["outT"].astype(np.float32).T
    out += bias[None, None, :]
    if run_kwargs:
        return out, res
    return out


# revision 33
# speedup vs baseline: 1.1837x; 1.0094x over previous
"""Multi-head causal attention (B=2, S=2048, D=1024, H=16, dh=64) on 8
Trainium2 NeuronCores.

Sharding: core i handles batch b = i//4 and head group g = i%4 (4 heads
each).  Per core everything is computed in a transposed layout:

  QT = Wq_g^T @ x_b^T          [256(hk), 2048(S)]   (bf16)
  KT = Wk_g^T @ x_b^T          [256(hk), 2048(S)]   (bf16)
  V  = x_b @ Wv_g              [2048(S), 4, 65]     (bf16; col 64 = ones)
  per chunk c (512 queries), head-pair hp, key block j (128 keys):
     scT[par] = KT_h[:,j]^T(lhsT) x QT_h[:,c]   -> PSUM [128, 2, 512]
                (the two heads of a pair use PE rows 0-63 / 64-127 and
                 run concurrently)
     expT     = exp(scT/8) (* causal mask when j >= 4c)        (bf16)
     zT_h    += V_aug[j]^T(lhsT) x expT[par]    -> PSUM [65, 512]
                (row 64 accumulates the softmax denominator s)
     ztn      = zT[0:64] * broadcast(1/s)       [256(hk), 2048] (bf16)
  outT = Wo_g^T(lhsT) x ztn                     [1024(d), 2048] (bf16)

Host: shards/transposes inputs, sums the 4 head-group partial outputs per
batch, adds b_O and the exact b_V fold (softmax rows sum to 1):
  out += b_O + sum_h b_V[h] @ W_O[h].
"""
import numpy as np
import ml_dtypes

import concourse.bacc as bacc
import concourse.mybir as mybir
import concourse.tile as tile
from concourse.bass_utils import run_bass_kernel_spmd

f32 = mybir.dt.float32
bf16 = mybir.dt.bfloat16
AF = mybir.ActivationFunctionType

B, S, D, H, DH = 2, 2048, 1024, 16, 64
NCORES = 8
HG = 4                # heads per core
HK = HG * DH          # 256
CH = 512              # query chunk
NCH = S // CH         # 4
KB = 128              # key block
DT = D // 128         # 8

_CACHE = {}


def _build_nc():
    nc = bacc.Bacc(None, target_bir_lowering=False, debug=False,
                   num_devices=NCORES)

    xt_d = nc.dram_tensor("xt", [128, DT, S], bf16, kind="ExternalInput")
    wq_d = nc.dram_tensor("wq", [128, DT, HK], bf16, kind="ExternalInput")
    wk_d = nc.dram_tensor("wk", [128, DT, HK], bf16, kind="ExternalInput")
    wv_d = nc.dram_tensor("wv", [128, DT, HK], bf16, kind="ExternalInput")
    wo_d = nc.dram_tensor("wo", [128, 2, D], bf16, kind="ExternalInput")
    bq_d = nc.dram_tensor("bq", [128, 2], f32, kind="ExternalInput")
    bk_d = nc.dram_tensor("bk", [128, 2], f32, kind="ExternalInput")
    mask_d = nc.dram_tensor("mask", [128, 4, 2, CH], bf16,
                            kind="ExternalInput")
    out_d = nc.dram_tensor("outT", [D, S], bf16, kind="ExternalOutput")

    with tile.TileContext(nc) as tc:
        with (
            tc.tile_pool(name="const", bufs=1) as cp,
            tc.tile_pool(name="big", bufs=1) as bp,
            tc.tile_pool(name="work", bufs=3) as wp,
            tc.tile_pool(name="psum", bufs=2, space="PSUM") as pp,
        ):
            # ---- loads
            wq = cp.tile([128, DT, HK], bf16)
            wk = cp.tile([128, DT, HK], bf16)
            wv = cp.tile([128, DT, HK], bf16)
            wo = cp.tile([128, 2, D], bf16)
            bq = cp.tile([128, 2], f32)
            bk = cp.tile([128, 2], f32)
            mask = cp.tile([128, 4, 2, CH], bf16)
            xt = bp.tile([128, DT, S], bf16)
            # xt streams on the HWDGE ring (sync); weights and constants go
            # through the SWDGE ring (gpsimd) so the two transfer in parallel
            # and the first projection chains can start as early as possible
            for di in range(DT):
                nc.sync.dma_start(wq[:, di, :], wq_d[:, di, :])
                nc.sync.dma_start(xt[:, di, 0:S // 2], xt_d[:, di, 0:S // 2])
            for di in range(DT):
                nc.sync.dma_start(xt[:, di, S // 2:], xt_d[:, di, S // 2:])
            nc.gpsimd.dma_start(wk, wk_d[:])
            nc.gpsimd.dma_start(wv, wv_d[:])
            nc.gpsimd.dma_start(bq, bq_d[:])
            nc.gpsimd.dma_start(bk, bk_d[:])
            nc.gpsimd.dma_start(mask, mask_d[:])
            nc.gpsimd.dma_start(wo, wo_d[:])

            qt = bp.tile([128, 2, S], bf16)
            kt = bp.tile([128, 2, S], bf16)
            # V padded to 128 columns (cols 65.. zero) so the zT matmul's
            # stationary is 128-wide -> fast weight load / ldw pipelining
            v = bp.tile([128, S // KB, HG, 128], bf16)
            ztn = bp.tile([128, 2, S], bf16)

            # ones column for the denominator trick; zero the pad
            nc.gpsimd.memset(v[:, :, :, DH:DH + 1], 1.0)
            nc.gpsimd.memset(v[:, :, :, DH + 1:], 0.0)

            # ---- phase B: projections (interleaved per chunk c).  During B
            # the attention psum tags are idle, so rotate B's psum tiles
            # through all tags to keep more projection chains in flight
            # while xt tiles stream in.
            btags = [("proj", 2), ("sc", 2), ("zt0", 1), ("zt1", 1)]
            bi = 0
            for c in range(NCH):
                cs = c * CH
                for m in range(2):
                    ps_q = pp.tile([128, CH], f32, tag=btags[bi % 4][0],
                                   bufs=btags[bi % 4][1], name=f"ps_q_{c}_{m}")
                    bi += 1
                    for di in range(DT):
                        nc.tensor.matmul(
                            ps_q, wq[:, di, m * 128:(m + 1) * 128],
                            xt[:, di, cs:cs + CH],
                            start=(di == 0), stop=(di == DT - 1))
                    nc.scalar.activation(qt[:, m, cs:cs + CH], ps_q,
                                         AF.Identity, bias=bq[:, m:m + 1])
                    ps_k = pp.tile([128, CH], f32, tag=btags[bi % 4][0],
                                   bufs=btags[bi % 4][1], name=f"ps_k_{c}_{m}")
                    bi += 1
                    for di in range(DT):
                        nc.tensor.matmul(
                            ps_k, wk[:, di, m * 128:(m + 1) * 128],
                            xt[:, di, cs:cs + CH],
                            start=(di == 0), stop=(di == DT - 1))
                    nc.scalar.activation(kt[:, m, cs:cs + CH], ps_k,
                                         AF.Identity, bias=bk[:, m:m + 1])
                for si in range(4 * c, 4 * c + 4):
                    ps_v = pp.tile([128, HG, DH], f32, tag=btags[bi % 4][0],
                                   bufs=btags[bi % 4][1], name=f"ps_v_{si}")
                    bi += 1
                    for di in range(DT):
                        nc.tensor.matmul(
                            ps_v, xt[:, di, si * KB:(si + 1) * KB],
                            wv[:, di, :],
                            start=(di == 0), stop=(di == DT - 1))
                    nc.vector.tensor_copy(v[:, si, :, 0:DH], ps_v)

            # ---- phase C: attention, head-pair row-packed scores
            for c in range(NCH):
                cs = c * CH
                nblk = 4 * c + 4       # key blocks for this chunk
                for hp in range(2):    # head pair (2hp, 2hp+1); m = hp
                    m = hp
                    zt0 = pp.tile([128, CH], f32, tag="zt0", bufs=1,
                                  name=f"zt0_{c}_{hp}")
                    zt1 = pp.tile([128, CH], f32, tag="zt1", bufs=1,
                                  name=f"zt1_{c}_{hp}")
                    zts = (zt0, zt1)
                    for j in range(nblk):
                        # diagonal blocks (t>=0): queries below 128t are
                        # fully masked -> compute only [128t, CH); the
                        # partially-masked region is just [128t, 128t+128)
                        t = j - 4 * c
                        ql = 128 * t if t > 0 else 0
                        sc = pp.tile([128, 2, CH], f32, tag="sc")
                        for par in range(2):
                            o = par * 64
                            nc.tensor.matmul(
                                sc[:, par, ql:],
                                kt[o:o + 64, m, j * KB:(j + 1) * KB],
                                qt[o:o + 64, m, cs + ql:cs + CH],
                                start=True, stop=True)
                        ex = wp.tile([128, 2, CH], bf16, tag="ex", bufs=6)
                        nc.scalar.activation(ex[:, :, ql:], sc[:, :, ql:],
                                             AF.Exp, scale=0.125)
                        if t >= 0:
                            qm = ql + 128
                            nc.vector.tensor_mul(ex[:, :, ql:qm],
                                                 ex[:, :, ql:qm],
                                                 mask[:, t, :, ql:qm])
                        for par in range(2):
                            h = 2 * hp + par
                            nc.tensor.matmul(
                                zts[par][:, ql:], v[:, j, h, :],
                                ex[:, par, ql:],
                                start=(j == 0), stop=(j == nblk - 1))
                    # normalize: ztn[h] = zt[0:64] / zt[64].  First copy
                    # the whole zT+denominator block to SBUF in one op so the
                    # PSUM accumulator frees immediately for the next head
                    # pair; the divide chain then runs entirely from SBUF.
                    for par in range(2):
                        h = 2 * hp + par
                        o = par * 64
                        zs = wp.tile([DH + 1, CH], f32, tag="zs", bufs=3,
                                     name=f"zs_{c}_{h}")
                        nc.vector.tensor_copy(zs, zts[par][0:DH + 1, :])
                        srow = wp.tile([1, CH], f32, tag="srow", bufs=3,
                                       name=f"srow_{c}_{h}")
                        nc.vector.tensor_copy(srow, zs[DH:DH + 1, :])
                        rec = wp.tile([1, CH], f32, tag="rec", bufs=3,
                                      name=f"rec_{c}_{h}")
                        nc.vector.reciprocal_approx_fast(rec, srow)
                        bc = wp.tile([64, CH], f32, tag="bc", bufs=3,
                                     name=f"bc_{c}_{h}")
                        nc.gpsimd.partition_broadcast(bc, rec)
                        nc.vector.tensor_mul(ztn[o:o + 64, m, cs:cs + CH],
                                             zs[0:DH, :], bc)

            # ---- phase D: output projection (emitted last; its matmuls
            # backfill PE idle slots during the ACT-paced tail of phase C).
            # The last chunk's tiles rotate through ALL psum tags: by then
            # phase C has released the sc/zt banks, so the final 8 output
            # tiles pipeline through 6 slots instead of 2.
            dtags = [("proj", 2), ("sc", 2), ("zt0", 1), ("zt1", 1)]
            for c in range(NCH):
                cs = c * CH
                for dt_i in range(DT):
                    tg, tb = dtags[dt_i % 4] if c == NCH - 1 else dtags[0]
                    ps_o = pp.tile([128, CH], f32, tag=tg, bufs=tb,
                                   name=f"ps_o_{c}_{dt_i}")
                    for m in range(2):
                        nc.tensor.matmul(
                            ps_o, wo[:, m, dt_i * 128:(dt_i + 1) * 128],
                            ztn[:, m, cs:cs + CH],
                            start=(m == 0), stop=(m == 1))
                    ost = wp.tile([128, CH], bf16, tag="ost", bufs=4)
                    nc.vector.tensor_copy(ost, ps_o)
                    nc.sync.dma_start(
                        out_d[dt_i * 128:(dt_i + 1) * 128, cs:cs + CH], ost)

    nc.compile()
    return nc


def _tile128(a, inner_shape):
    """[N*128, ...] -> [128, N, ...] partition-major layout."""
    n = a.shape[0] // 128
    return np.ascontiguousarray(
        a.reshape((n, 128) + a.shape[1:]).swapaxes(0, 1)).reshape(
            (128, n) + inner_shape)


def _prep_core(x, W_Q, W_K, W_V, W_O, b_Q, b_K, b, g):
    hs = slice(g * HG, (g + 1) * HG)
    bfl = ml_dtypes.bfloat16

    xtp = np.ascontiguousarray(x[b].T)                       # [D, S]
    xt = _tile128(xtp, (S,)).astype(bfl)                     # [128, DT, S]

    def prep_w(w):                                           # [H,D,dh] slice
        wc = np.ascontiguousarray(
            w[hs].transpose(1, 0, 2).reshape(D, HK))         # [D, HK]
        return _tile128(wc, (HK,)).astype(bfl)               # [128, DT, HK]

    wq, wk, wv = prep_w(W_Q), prep_w(W_K), prep_w(W_V)
    woc = W_O[hs].reshape(HK, D)                             # [HK, D]
    wo = _tile128(woc, (D,)).astype(bfl)                     # [128, 2, D]

    bq = np.ascontiguousarray(
        b_Q[hs].reshape(HK).reshape(2, 128).T).astype(np.float32)
    bk = np.ascontiguousarray(
        b_K[hs].reshape(HK).reshape(2, 128).T).astype(np.float32)

    r = np.arange(128)[:, None, None]
    f = np.arange(CH)[None, None, :]
    t = np.arange(4)[None, :, None]
    m3 = (f >= r + 128 * t)                                  # [128, 4, CH]
    mask = np.repeat(m3[:, :, None, :], 2, axis=2).astype(bfl)

    return {"xt": xt, "wq": wq, "wk": wk, "wv": wv, "wo": wo,
            "bq": bq, "bk": bk, "mask": mask}


def kernel(x, W_Q, W_K, W_V, W_O, b_Q, b_K, b_V, b_O, **run_kwargs):
    x = np.asarray(x, dtype=np.float32)
    W_Q = np.asarray(W_Q, dtype=np.float32)
    W_K = np.asarray(W_K, dtype=np.float32)
    W_V = np.asarray(W_V, dtype=np.float32)
    W_O = np.asarray(W_O, dtype=np.float32)
    b_Q = np.asarray(b_Q, dtype=np.float32)
    b_K = np.asarray(b_K, dtype=np.float32)
    b_V = np.asarray(b_V, dtype=np.float32)
    b_O = np.asarray(b_O, dtype=np.float32)

    if "nc" not in _CACHE:
        _CACHE["nc"] = _build_nc()
    nc = _CACHE["nc"]

    in_maps = []
    for i in range(NCORES):
        b, g = i // HG, i % HG
        in_maps.append(_prep_core(x, W_Q, W_K, W_V, W_O, b_Q, b_K, b, g))

    res = run_bass_kernel_spmd(nc, in_maps, core_ids=list(range(NCORES)),
                               **run_kwargs)

    # exact fold of b_V through W_O (softmax rows sum to 1), plus b_O
    bias = (b_O.astype(np.float64)
            + b_V.reshape(H * DH).astype(np.float64)
            @ W_O.reshape(H * DH, D).astype(np.float64)).astype(np.float32)

    out = np.zeros((B, S, D), dtype=np.float32)
    for i in range(NCORES):
        b = i // HG
        out[b] += res.results[i]["outT"].astype(np.float32).T
    out += bias[None, None, :]
    if run_kwargs:
        return out, res
    return out


# revision 34
# speedup vs baseline: 1.1934x; 1.0082x over previous
"""Multi-head causal attention (B=2, S=2048, D=1024, H=16, dh=64) on 8
Trainium2 NeuronCores.

Sharding: core i handles batch b = i//4 and head group g = i%4 (4 heads
each).  Per core everything is computed in a transposed layout:

  QT = Wq_g^T @ x_b^T          [256(hk), 2048(S)]   (bf16)
  KT = Wk_g^T @ x_b^T          [256(hk), 2048(S)]   (bf16)
  V  = x_b @ Wv_g              [2048(S), 4, 65]     (bf16; col 64 = ones)
  per chunk c (512 queries), head-pair hp, key block j (128 keys):
     scT[par] = KT_h[:,j]^T(lhsT) x QT_h[:,c]   -> PSUM [128, 2, 512]
                (the two heads of a pair use PE rows 0-63 / 64-127 and
                 run concurrently)
     expT     = exp(scT/8) (* causal mask when j >= 4c)        (bf16)
     zT_h    += V_aug[j]^T(lhsT) x expT[par]    -> PSUM [65, 512]
                (row 64 accumulates the softmax denominator s)
     ztn      = zT[0:64] * broadcast(1/s)       [256(hk), 2048] (bf16)
  outT = Wo_g^T(lhsT) x ztn                     [1024(d), 2048] (bf16)

Host: shards/transposes inputs, sums the 4 head-group partial outputs per
batch, adds b_O and the exact b_V fold (softmax rows sum to 1):
  out += b_O + sum_h b_V[h] @ W_O[h].
"""
import numpy as np
import ml_dtypes

import concourse.bacc as bacc
import concourse.mybir as mybir
import concourse.tile as tile
from concourse.bass_utils import run_bass_kernel_spmd

f32 = mybir.dt.float32
bf16 = mybir.dt.bfloat16
AF = mybir.ActivationFunctionType

B, S, D, H, DH = 2, 2048, 1024, 16, 64
NCORES = 8
HG = 4                # heads per core
HK = HG * DH          # 256
CH = 512              # query chunk
NCH = S // CH         # 4
KB = 128              # key block
DT = D // 128         # 8

_CACHE = {}


def _build_nc():
    nc = bacc.Bacc(None, target_bir_lowering=False, debug=False,
                   num_devices=NCORES)

    xt_d = nc.dram_tensor("xt", [128, DT, S], bf16, kind="ExternalInput")
    wq_d = nc.dram_tensor("wq", [128, DT, HK], bf16, kind="ExternalInput")
    wk_d = nc.dram_tensor("wk", [128, DT, HK], bf16, kind="ExternalInput")
    wv_d = nc.dram_tensor("wv", [128, DT, HK], bf16, kind="ExternalInput")
    wo_d = nc.dram_tensor("wo", [128, 2, D], bf16, kind="ExternalInput")
    bq_d = nc.dram_tensor("bq", [128, 2], f32, kind="ExternalInput")
    bk_d = nc.dram_tensor("bk", [128, 2], f32, kind="ExternalInput")
    mask_d = nc.dram_tensor("mask", [128, 4, 2, CH], bf16,
                            kind="ExternalInput")
    out_d = nc.dram_tensor("outT", [D, S], bf16, kind="ExternalOutput")

    with tile.TileContext(nc) as tc:
        with (
            tc.tile_pool(name="const", bufs=1) as cp,
            tc.tile_pool(name="big", bufs=1) as bp,
            tc.tile_pool(name="work", bufs=3) as wp,
            tc.tile_pool(name="psum", bufs=2, space="PSUM") as pp,
        ):
            # ---- loads
            wq = cp.tile([128, DT, HK], bf16)
            wk = cp.tile([128, DT, HK], bf16)
            wv = cp.tile([128, DT, HK], bf16)
            wo = cp.tile([128, 2, D], bf16)
            bq = cp.tile([128, 2], f32)
            bk = cp.tile([128, 2], f32)
            mask = cp.tile([128, 4, 2, CH], bf16)
            xt = bp.tile([128, DT, S], bf16)
            # xt streams on the HWDGE ring (sync); weights and constants go
            # through the SWDGE ring (gpsimd) so the two transfer in parallel
            # and the first projection chains can start as early as possible
            for di in range(DT):
                nc.sync.dma_start(wq[:, di, :], wq_d[:, di, :])
                nc.sync.dma_start(xt[:, di, 0:S // 2], xt_d[:, di, 0:S // 2])
            for di in range(DT):
                nc.sync.dma_start(xt[:, di, S // 2:], xt_d[:, di, S // 2:])
            nc.gpsimd.dma_start(wk, wk_d[:])
            nc.gpsimd.dma_start(wv, wv_d[:])
            nc.gpsimd.dma_start(bq, bq_d[:])
            nc.gpsimd.dma_start(bk, bk_d[:])
            nc.gpsimd.dma_start(mask, mask_d[:])
            nc.gpsimd.dma_start(wo, wo_d[:])

            qt = bp.tile([128, 2, S], bf16)
            kt = bp.tile([128, 2, S], bf16)
            # V padded to 128 columns (cols 65.. zero) so the zT matmul's
            # stationary is 128-wide -> fast weight load / ldw pipelining
            v = bp.tile([128, S // KB, HG, 128], bf16)
            ztn = bp.tile([128, 2, S], bf16)

            # ones column for the denominator trick; zero the pad
            nc.gpsimd.memset(v[:, :, :, DH:DH + 1], 1.0)
            nc.gpsimd.memset(v[:, :, :, DH + 1:], 0.0)

            # ---- phase B: projections (interleaved per chunk c).  During B
            # the attention psum tags are idle, so rotate B's psum tiles
            # through all tags to keep more projection chains in flight
            # while xt tiles stream in.
            btags = [("proj", 2), ("sc", 2), ("zt0", 1), ("zt1", 1)]
            bi = 0
            for c in range(NCH):
                cs = c * CH
                for m in range(2):
                    ps_q = pp.tile([128, CH], f32, tag=btags[bi % 4][0],
                                   bufs=btags[bi % 4][1], name=f"ps_q_{c}_{m}")
                    bi += 1
                    for di in range(DT):
                        nc.tensor.matmul(
                            ps_q, wq[:, di, m * 128:(m + 1) * 128],
                            xt[:, di, cs:cs + CH],
                            start=(di == 0), stop=(di == DT - 1))
                    nc.scalar.activation(qt[:, m, cs:cs + CH], ps_q,
                                         AF.Identity, bias=bq[:, m:m + 1])
                    ps_k = pp.tile([128, CH], f32, tag=btags[bi % 4][0],
                                   bufs=btags[bi % 4][1], name=f"ps_k_{c}_{m}")
                    bi += 1
                    for di in range(DT):
                        nc.tensor.matmul(
                            ps_k, wk[:, di, m * 128:(m + 1) * 128],
                            xt[:, di, cs:cs + CH],
                            start=(di == 0), stop=(di == DT - 1))
                    nc.scalar.activation(kt[:, m, cs:cs + CH], ps_k,
                                         AF.Identity, bias=bk[:, m:m + 1])
                for si in range(4 * c, 4 * c + 4):
                    ps_v = pp.tile([128, HG, DH], f32, tag=btags[bi % 4][0],
                                   bufs=btags[bi % 4][1], name=f"ps_v_{si}")
                    bi += 1
                    for di in range(DT):
                        nc.tensor.matmul(
                            ps_v, xt[:, di, si * KB:(si + 1) * KB],
                            wv[:, di, :],
                            start=(di == 0), stop=(di == DT - 1))
                    nc.vector.tensor_copy(v[:, si, :, 0:DH], ps_v)

            # ---- phase C: attention, head-pair row-packed scores
            for c in range(NCH):
                cs = c * CH
                nblk = 4 * c + 4       # key blocks for this chunk
                for hp in range(2):    # head pair (2hp, 2hp+1); m = hp
                    m = hp
                    zt0 = pp.tile([128, CH], f32, tag="zt0", bufs=1,
                                  name=f"zt0_{c}_{hp}")
                    zt1 = pp.tile([128, CH], f32, tag="zt1", bufs=1,
                                  name=f"zt1_{c}_{hp}")
                    zts = (zt0, zt1)
                    for j in range(nblk):
                        # diagonal blocks (t>=0): queries below 128t are
                        # fully masked -> compute only [128t, CH); the
                        # partially-masked region is just [128t, 128t+128)
                        t = j - 4 * c
                        ql = 128 * t if t > 0 else 0
                        sc = pp.tile([128, 2, CH], f32, tag="sc")
                        for par in range(2):
                            o = par * 64
                            nc.tensor.matmul(
                                sc[:, par, ql:],
                                kt[o:o + 64, m, j * KB:(j + 1) * KB],
                                qt[o:o + 64, m, cs + ql:cs + CH],
                                start=True, stop=True)
                        ex = wp.tile([128, 2, CH], bf16, tag="ex", bufs=6)
                        nc.scalar.activation(ex[:, :, ql:], sc[:, :, ql:],
                                             AF.Exp, scale=0.125)
                        if t >= 0:
                            qm = ql + 128
                            nc.vector.tensor_mul(ex[:, :, ql:qm],
                                                 ex[:, :, ql:qm],
                                                 mask[:, t, :, ql:qm])
                        for par in range(2):
                            h = 2 * hp + par
                            nc.tensor.matmul(
                                zts[par][:, ql:], v[:, j, h, :],
                                ex[:, par, ql:],
                                start=(j == 0), stop=(j == nblk - 1))
                    # normalize: ztn[h] = zt[0:64] / zt[64].  First copy
                    # the whole zT+denominator block to SBUF in one op so the
                    # PSUM accumulator frees immediately for the next head
                    # pair; the divide chain then runs entirely from SBUF.
                    # The very last pair has no successor wanting its PSUM
                    # slot, so skip the bounce there - it sits on the final
                    # serial tail before the last output projection.
                    last = (c == NCH - 1 and hp == 1)
                    for par in range(2):
                        h = 2 * hp + par
                        o = par * 64
                        if last:
                            zsrc = zts[par]
                        else:
                            zs = wp.tile([DH + 1, CH], f32, tag="zs", bufs=3,
                                         name=f"zs_{c}_{h}")
                            nc.vector.tensor_copy(zs, zts[par][0:DH + 1, :])
                            zsrc = zs
                        srow = wp.tile([1, CH], f32, tag="srow", bufs=3,
                                       name=f"srow_{c}_{h}")
                        nc.vector.tensor_copy(srow, zsrc[DH:DH + 1, :])
                        rec = wp.tile([1, CH], f32, tag="rec", bufs=3,
                                      name=f"rec_{c}_{h}")
                        nc.vector.reciprocal_approx_fast(rec, srow)
                        bc = wp.tile([64, CH], f32, tag="bc", bufs=3,
                                     name=f"bc_{c}_{h}")
                        nc.gpsimd.partition_broadcast(bc, rec)
                        nc.vector.tensor_mul(ztn[o:o + 64, m, cs:cs + CH],
                                             zsrc[0:DH, :], bc)

            # ---- phase D: output projection (emitted last; its matmuls
            # backfill PE idle slots during the ACT-paced tail of phase C).
            # The last chunk's tiles rotate through ALL psum tags: by then
            # phase C has released the sc/zt banks, so the final 8 output
            # tiles pipeline through 6 slots instead of 2.
            dtags = [("proj", 2), ("sc", 2), ("zt0", 1), ("zt1", 1)]
            for c in range(NCH):
                cs = c * CH
                for dt_i in range(DT):
                    tg, tb = dtags[dt_i % 4] if c == NCH - 1 else dtags[0]
                    ps_o = pp.tile([128, CH], f32, tag=tg, bufs=tb,
                                   name=f"ps_o_{c}_{dt_i}")
                    for m in range(2):
                        nc.tensor.matmul(
                            ps_o, wo[:, m, dt_i * 128:(dt_i + 1) * 128],
                            ztn[:, m, cs:cs + CH],
                            start=(m == 0), stop=(m == 1))
                    ost = wp.tile([128, CH], bf16, tag="ost", bufs=4)
                    nc.vector.tensor_copy(ost, ps_o)
                    nc.sync.dma_start(
                        out_d[dt_i * 128:(dt_i + 1) * 128, cs:cs + CH], ost)

    nc.compile()
    return nc


def _tile128(a, inner_shape):
    """[N*128, ...] -> [128, N, ...] partition-major layout."""
    n = a.shape[0] // 128
    return np.ascontiguousarray(
        a.reshape((n, 128) + a.shape[1:]).swapaxes(0, 1)).reshape(
            (128, n) + inner_shape)


def _prep_core(x, W_Q, W_K, W_V, W_O, b_Q, b_K, b, g):
    hs = slice(g * HG, (g + 1) * HG)
    bfl = ml_dtypes.bfloat16

    xtp = np.ascontiguousarray(x[b].T)                       # [D, S]
    xt = _tile128(xtp, (S,)).astype(bfl)                     # [128, DT, S]

    def prep_w(w):                                           # [H,D,dh] slice
        wc = np.ascontiguousarray(
            w[hs].transpose(1, 0, 2).reshape(D, HK))         # [D, HK]
        return _tile128(wc, (HK,)).astype(bfl)               # [128, DT, HK]

    wq, wk, wv = prep_w(W_Q), prep_w(W_K), prep_w(W_V)
    woc = W_O[hs].reshape(HK, D)                             # [HK, D]
    wo = _tile128(woc, (D,)).astype(bfl)                     # [128, 2, D]

    bq = np.ascontiguousarray(
        b_Q[hs].reshape(HK).reshape(2, 128).T).astype(np.float32)
    bk = np.ascontiguousarray(
        b_K[hs].reshape(HK).reshape(2, 128).T).astype(np.float32)

    r = np.arange(128)[:, None, None]
    f = np.arange(CH)[None, None, :]
    t = np.arange(4)[None, :, None]
    m3 = (f >= r + 128 * t)                                  # [128, 4, CH]
    mask = np.repeat(m3[:, :, None, :], 2, axis=2).astype(bfl)

    return {"xt": xt, "wq": wq, "wk": wk, "wv": wv, "wo": wo,
            "bq": bq, "bk": bk, "mask": mask}


def kernel(x, W_Q, W_K, W_V, W_O, b_Q, b_K, b_V, b_O, **run_kwargs):
    x = np.asarray(x, dtype=np.float32)
    W_Q = np.asarray(W_Q, dtype=np.float32)
    W_K = np.asarray(W_K, dtype=np.float32)
    W_V = np.asarray(W_V, dtype=np.float32)
    W_O = np.asarray(W_O, dtype=np.float32)
    b_Q = np.asarray(b_Q, dtype=np.float32)
    b_K = np.asarray(b_K, dtype=np.float32)
    b_V = np.asarray(b_V, dtype=np.float32)
    b_O = np.asarray(b_O, dtype=np.float32)

    if "nc" not in _CACHE:
        _CACHE["nc"] = _build_nc()
    nc = _CACHE["nc"]

    in_maps = []
    for i in range(NCORES):
        b, g = i // HG, i % HG
        in_maps.append(_prep_core(x, W_Q, W_K, W_V, W_O, b_Q, b_K, b, g))

    res = run_bass_kernel_spmd(nc, in_maps, core_ids=list(range(NCORES)),
                               **run_kwargs)

    # exact fold of b_V through W_O (softmax rows sum to 1), plus b_O
    bias = (b_O.astype(np.float64)
            + b_V.reshape(H * DH).astype(np.float64)
            @ W_O.reshape(H * DH, D).astype(np.float64)).astype(np.float32)

    out = np.zeros((B, S, D), dtype=np.float32)
    for i in range(NCORES):
        b = i // HG
        out[b] += res.results[i]["outT"].astype(np.float32).T
    out += bias[None, None, :]
    if run_kwargs:
        return out, res
    return out
